# revision 2
# baseline (speedup 1.0000x reference)
"""Gaussian RBF kernel for Trainium2, data-parallel over batch across 8 cores.

exp(-0.5*||x-mu||^2/sigma^2) folded into ONE augmented GEMM + Exp:
  E[s,o] = sum_d x[s,d]*(2*a[o]*mus[o,d]) + x2[s]*(-a[o]) + 1*(-a[o]*m2[o])
with a = 0.5/sigma^2.  Augmented contraction K = D+2 = 66; the tiny weight
matrix W (66,512) and the x2/ones augmentation are built on host.
Per core: xaT (66,4096) @ W -> (4096,512), Exp on ACT, DMA out.

Raw bass engine programs (explicit semaphores) — the Tile framework's
attached-wait sync scheme trips "Too many sync wait commands" in this
compiler build, so engines are programmed directly.
"""
import numpy as np
from concourse import bass, mybir
from concourse import bass_utils

B, S, D, O = 8, 4096, 64, 512
K = D + 2          # 66: [x, x2, 1]
P = 128            # rows (s) per tile
NT = S // P        # 32 tiles
NPS = 4            # psum banks in rotation
NB = 4             # sbuf output buffers

FP = mybir.dt.float32


def _build():
    nc = bass.Bass()
    xaT = nc.declare_dram_parameter("xaT", [K, S], FP, isOutput=False)
    w = nc.declare_dram_parameter("w", [K, O], FP, isOutput=False)
    out = nc.declare_dram_parameter("out", [S, O], FP, isOutput=True)

    with (
        nc.sbuf_tensor([K, S], FP) as xt,
        nc.sbuf_tensor([K, O], FP) as wt,
        nc.sbuf_tensor([P, NB * O], FP) as ot,
        nc.psum_tensor([P, NPS * O], FP) as ps,
        nc.Block() as block,
        nc.semaphore("dma_in") as dma_in,
        nc.semaphore("mm") as mm,
        nc.semaphore("act_s") as act_s,
        nc.semaphore("dma_out") as dma_out,
    ):
        @block.sync
        def _(sync):
            sync.dma_start(out=wt[:], in_=w[:]).then_inc(dma_in, 16)
            sync.dma_start(out=xt[:], in_=xaT[:]).then_inc(dma_in, 16)
            for t in range(NT):
                sync.wait_ge(act_s, t + 1)
                sync.dma_start(
                    out=out[t * P:(t + 1) * P, :],
                    in_=ot[:, (t % NB) * O:(t % NB + 1) * O],
                ).then_inc(dma_out, 16)
            sync.wait_ge(dma_out, 16 * NT)

        @block.tensor
        def _(pe):  # tensor engine
            pe.wait_ge(dma_in, 32)
            for t in range(NT):
                if t >= NPS:
                    pe.wait_ge(act_s, t - NPS + 1)
                pe.matmul(
                    ps[:, (t % NPS) * O:(t % NPS + 1) * O],
                    xt[:, t * P:(t + 1) * P],
                    wt[:],
                    start=True,
                    stop=True,
                ).then_inc(mm, 1)

        @block.scalar
        def _(scalar):
            for t in range(NT):
                scalar.wait_ge(mm, t + 1)
                if t >= NB:
                    scalar.wait_ge(dma_out, 16 * (t - NB + 1))
                scalar.activation(
                    ot[:, (t % NB) * O:(t % NB + 1) * O],
                    ps[:, (t % NPS) * O:(t % NPS + 1) * O],
                    mybir.ActivationFunctionType.Exp,
                ).then_inc(act_s, 1)

    return nc


def kernel(x, mus, log_sigmas):
    x = np.asarray(x, np.float32)
    mus = np.asarray(mus, np.float32)
    log_sigmas = np.asarray(log_sigmas, np.float32)

    a = 0.5 * np.exp(-2.0 * log_sigmas.astype(np.float64))          # (O,)
    m2 = np.sum(mus.astype(np.float64) ** 2, axis=1)                # (O,)
    W = np.empty((K, O), np.float32)
    W[:D] = (2.0 * a[None, :] * mus.T.astype(np.float64)).astype(np.float32)
    W[D] = (-a).astype(np.float32)
    W[D + 1] = (-a * m2).astype(np.float32)

    x2 = np.sum(x * x, axis=-1)                                     # (B,S)
    in_maps = []
    for i in range(B):
        xa = np.empty((S, K), np.float32)
        xa[:, :D] = x[i]
        xa[:, D] = x2[i]
        xa[:, D + 1] = 1.0
        in_maps.append({"xaT": np.ascontiguousarray(xa.T), "w": W})

    nc = _build()
    global LAST_RESULT
    LAST_RESULT = res = bass_utils.run_bass_kernel_spmd(
        nc, in_maps, list(range(B)), **RUN_KWARGS
    )
    return np.stack([r["out"] for r in res.results], axis=0)


LAST_RESULT = None
RUN_KWARGS: dict = {}



# revision 3
# speedup vs baseline: 2.1051x; 2.1051x over previous
"""Gaussian RBF kernel for Trainium2, data-parallel over batch across 8 cores.

exp(-0.5*||x-mu||^2/sigma^2) folded into ONE augmented GEMM + Exp:
  E[s,o] = sum_d x[s,d]*(2*a[o]*mus[o,d]) + x2[s]*(-a[o]) + 1*(-a[o]*m2[o])
with a = 0.5/sigma^2.  Augmented contraction K = D+2 = 66; the tiny weight
matrix W (66,512) and the x2/ones augmentation are built on host.

Per core: xaT (66,4096) @ W -> (4096,512), Exp on ACT engine, DMA out.

Performance structure (vs the naive version):
- fp32r matmuls (1 cycle/row at 512-wide moving dim, vs 4 for fp32)
- bf16 output store (halves the dominant HBM write traffic; bf16 keeps
  f32's exponent range so underflow behaviour matches the reference)
- input streamed in 8 chunks so matmuls start early
- activations cover 4 PSUM banks each (amortize PSUM access + seq
  overhead), ping-pong across the 8 banks
- output DMAs batched 4 tiles (512 rows) per instruction to amortize the
  ~565ns DGE config time per DMA

Raw bass engine programs (explicit semaphores) — the Tile framework's
attached-wait sync scheme trips "Too many sync wait commands" in this
compiler build, so engines are programmed directly.
"""
import numpy as np
from concourse import bass, mybir
from concourse import bass_utils

B, S, D, O = 8, 4096, 64, 512
K = D + 2            # 66: [x, x2, 1]
P = 128              # rows (s) per tile
NT = S // P          # 32 tiles
TPA = 4              # tiles per activation instruction (4 PSUM banks)
NG = NT // TPA       # 8 activation groups == 8 output DMAs
NCH = 8              # input chunks (each 512 cols == 4 tiles)
TPC = NT // NCH      # tiles per input chunk

FP = mybir.dt.float32
FR = mybir.dt.float32r
BF = mybir.dt.bfloat16


def _build():
    nc = bass.Bass()
    xaT = nc.declare_dram_parameter("xaT", [K, S], FR, isOutput=False)
    w = nc.declare_dram_parameter("w", [K, O], FR, isOutput=False)
    out = nc.declare_dram_parameter("out", [S, O], BF, isOutput=True)

    with (
        nc.sbuf_tensor([K, S], FR) as xt,
        nc.sbuf_tensor([K, O], FR) as wt,
        nc.sbuf_tensor([P, NT * O], BF) as ot,
        nc.psum_tensor([P, 8 * O], FP) as ps,
        nc.Block() as block,
        nc.semaphore("sem_w") as sem_w,
        nc.semaphore("sx0") as sx0,
        nc.semaphore("sx1") as sx1,
        nc.semaphore("sx2") as sx2,
        nc.semaphore("sx3") as sx3,
        nc.semaphore("sx4") as sx4,
        nc.semaphore("sx5") as sx5,
        nc.semaphore("sx6") as sx6,
        nc.semaphore("sx7") as sx7,
        nc.semaphore("mm") as mm,
        nc.semaphore("act_s") as act_s,
        nc.semaphore("dma_out") as dma_out,
    ):
        sx = [sx0, sx1, sx2, sx3, sx4, sx5, sx6, sx7]

        @block.sync
        def _(sync):
            cw = S // NCH
            for c in range(NCH):
                sync.dma_start(
                    out=xt[:, c * cw:(c + 1) * cw], in_=xaT[:, c * cw:(c + 1) * cw]
                ).then_inc(sx[c], 16)
            dst = out[:].rearrange("(g t p) o -> g p t o", g=NG, t=TPA, p=P)
            for g in range(NG):
                sync.wait_ge(act_s, g + 1)
                src = ot[:, g * TPA * O:(g + 1) * TPA * O]
                sync.dma_start(
                    out=dst[g], in_=src.rearrange("p (t o) -> p t o", t=TPA, o=O)
                ).then_inc(dma_out, 16)
            sync.wait_ge(dma_out, 16 * NG)

        @block.scalar
        def _(scalar):
            # W load issued from the ACT engine's HWDGE ring so it runs in
            # parallel with the SP ring's first x chunk.
            scalar.dma_start(out=wt[:], in_=w[:]).then_inc(sem_w, 16)
            for j in range(NG):
                scalar.wait_ge(mm, TPA * (j + 1))
                scalar.activation(
                    ot[:, j * TPA * O:(j + 1) * TPA * O],
                    ps[:, (j % 2) * TPA * O:((j % 2) + 1) * TPA * O],
                    mybir.ActivationFunctionType.Exp,
                ).then_inc(act_s, 1)

        @block.tensor
        def _(pe):
            pe.wait_ge(sem_w, 16)
            for t in range(NT):
                if t % TPC == 0:
                    pe.wait_ge(sx[t // TPC], 16)
                if t >= 8 and (t - 8) % TPA == 0:
                    # PSUM bank t%8 is recycled; freed by act (t-8)//TPA
                    pe.wait_ge(act_s, (t - 8) // TPA + 1)
                pe.matmul(
                    ps[:, (t % 8) * O:(t % 8 + 1) * O],
                    xt[:, t * P:(t + 1) * P],
                    wt[:],
                    start=True,
                    stop=True,
                ).then_inc(mm, 1)

    return nc


def kernel(x, mus, log_sigmas):
    x = np.asarray(x, np.float32)
    mus = np.asarray(mus, np.float32)
    log_sigmas = np.asarray(log_sigmas, np.float32)

    a = 0.5 * np.exp(-2.0 * log_sigmas.astype(np.float64))          # (O,)
    m2 = np.sum(mus.astype(np.float64) ** 2, axis=1)                # (O,)
    W = np.empty((K, O), np.float32)
    W[:D] = (2.0 * a[None, :] * mus.T.astype(np.float64)).astype(np.float32)
    W[D] = (-a).astype(np.float32)
    W[D + 1] = (-a * m2).astype(np.float32)

    x2 = np.sum(x * x, axis=-1)                                     # (B,S)
    in_maps = []
    for i in range(B):
        xa = np.empty((S, K), np.float32)
        xa[:, :D] = x[i]
        xa[:, D] = x2[i]
        xa[:, D + 1] = 1.0
        in_maps.append({"xaT": np.ascontiguousarray(xa.T), "w": W})

    nc = _build()
    global LAST_RESULT
    LAST_RESULT = res = bass_utils.run_bass_kernel_spmd(
        nc, in_maps, list(range(B)), **RUN_KWARGS
    )
    return np.stack([r["out"].astype(np.float32) for r in res.results], axis=0)


LAST_RESULT = None
RUN_KWARGS: dict = {}


# revision 4
# speedup vs baseline: 2.1888x; 1.0398x over previous
"""Gaussian RBF kernel for Trainium2, data-parallel over batch across 8 cores.

exp(-0.5*||x-mu||^2/sigma^2) folded into ONE augmented GEMM + Exp:
  E[s,o] = sum_d x[s,d]*(2*a[o]*mus[o,d]) + x2[s]*(-a[o]) + 1*(-a[o]*m2[o])
with a = 0.5/sigma^2.  Augmented contraction K = D+2 = 66; the tiny weight
matrix W (66,512) and the x2/ones augmentation are built on host.

Per core: xaT (66,4096) @ W -> (4096,512), Exp on ACT engine, DMA out.

Performance structure (vs the naive version):
- fp32r matmuls (1 cycle/row at 512-wide moving dim, vs 4 for fp32)
- bf16 output store (halves the dominant HBM write traffic; bf16 keeps
  f32's exponent range so underflow behaviour matches the reference)
- input streamed in 8 chunks so matmuls start early
- activations cover 4 PSUM banks each (amortize PSUM access + seq
  overhead), ping-pong across the 8 banks
- output DMAs batched 4 tiles (512 rows) per instruction to amortize the
  ~565ns DGE config time per DMA

Raw bass engine programs (explicit semaphores) — the Tile framework's
attached-wait sync scheme trips "Too many sync wait commands" in this
compiler build, so engines are programmed directly.
"""
import numpy as np
from concourse import bass, mybir
from concourse import bass_utils

B, S, D, O = 8, 4096, 64, 512
K = D + 2            # 66: [x, x2, 1]
P = 128              # rows (s) per tile
NT = S // P          # 32 tiles
TPA = 4              # tiles per activation instruction (4 PSUM banks)
NG = NT // TPA       # 8 activation groups == 8 output DMAs
NCH = 8              # input chunks (each 512 cols == 4 tiles)
TPC = NT // NCH      # tiles per input chunk

FP = mybir.dt.float32
FR = mybir.dt.float32r
BF = mybir.dt.bfloat16


def _build():
    nc = bass.Bass()
    xaT = nc.declare_dram_parameter("xaT", [K, S], FR, isOutput=False)
    w = nc.declare_dram_parameter("w", [K, O], FR, isOutput=False)
    out = nc.declare_dram_parameter("out", [S, O], BF, isOutput=True)

    with (
        nc.sbuf_tensor([K, S], FR) as xt,
        nc.sbuf_tensor([K, O], FR) as wt,
        nc.sbuf_tensor([K, 640], FR) as warm,
        nc.sbuf_tensor([P, O], BF) as scratch,
        nc.sbuf_tensor([P, NT * O], BF) as ot,
        nc.psum_tensor([P, 8 * O], FP) as ps,
        nc.Block() as block,
        nc.semaphore("sem_w") as sem_w,
        nc.semaphore("sx0") as sx0,
        nc.semaphore("sx1") as sx1,
        nc.semaphore("sx2") as sx2,
        nc.semaphore("sx3") as sx3,
        nc.semaphore("sx4") as sx4,
        nc.semaphore("sx5") as sx5,
        nc.semaphore("sx6") as sx6,
        nc.semaphore("sx7") as sx7,
        nc.semaphore("mm") as mm,
        nc.semaphore("act_s") as act_s,
        nc.semaphore("dma_out") as dma_out,
    ):
        sx = [sx0, sx1, sx2, sx3, sx4, sx5, sx6, sx7]

        @block.sync
        def _(sync):
            # W first: tiny, and every matmul needs it.
            sync.dma_start(out=wt[:], in_=w[:]).then_inc(sem_w, 16)
            cw = S // NCH
            for c in range(NCH):
                sync.dma_start(
                    out=xt[:, c * cw:(c + 1) * cw], in_=xaT[:, c * cw:(c + 1) * cw]
                ).then_inc(sx[c], 16)
            dst = out[:].rearrange("(g t p) o -> g p t o", g=NG, t=TPA, p=P)
            for g in range(NG):
                sync.wait_ge(act_s, g + 2)
                src = ot[:, g * TPA * O:(g + 1) * TPA * O]
                sync.dma_start(
                    out=dst[g], in_=src.rearrange("p (t o) -> p t o", t=TPA, o=O)
                ).then_inc(dma_out, 16)
            sync.wait_ge(dma_out, 16 * NG)

        @block.scalar
        def _(scalar):
            # Warm-up activation (reads stale PSUM bank 6, writes scratch):
            # spins up the ACT sequencer/datapath while input DMAs fly.
            scalar.activation(
                scratch[:],
                ps[:, 6 * O:7 * O],
                mybir.ActivationFunctionType.Exp,
            ).then_inc(act_s, 1)
            for j in range(NG):
                scalar.wait_ge(mm, TPA * (j + 1))
                scalar.activation(
                    ot[:, j * TPA * O:(j + 1) * TPA * O],
                    ps[:, (j % 2) * TPA * O:((j % 2) + 1) * TPA * O],
                    mybir.ActivationFunctionType.Exp,
                ).then_inc(act_s, 1)

        @block.tensor
        def _(pe):
            # Warm-up matmul on a never-written scratch tensor: starts the PE
            # pipeline/clock while the input DMAs are still in flight.
            pe.matmul(
                ps[:, 7 * O:8 * O],
                warm[:, 0:P],
                warm[:, P:P + O],
                start=True,
                stop=True,
            )
            pe.wait_ge(sem_w, 16)
            for t in range(NT):
                if t % TPC == 0:
                    pe.wait_ge(sx[t // TPC], 16)
                if t >= 8 and (t - 8) % TPA == 0:
                    # PSUM bank t%8 is recycled; freed by act (t-8)//TPA
                    # (act_s is offset by 1 for the warm-up activation)
                    pe.wait_ge(act_s, (t - 8) // TPA + 2)
                pe.matmul(
                    ps[:, (t % 8) * O:(t % 8 + 1) * O],
                    xt[:, t * P:(t + 1) * P],
                    wt[:],
                    start=True,
                    stop=True,
                ).then_inc(mm, 1)

    return nc


def kernel(x, mus, log_sigmas):
    x = np.asarray(x, np.float32)
    mus = np.asarray(mus, np.float32)
    log_sigmas = np.asarray(log_sigmas, np.float32)

    a = 0.5 * np.exp(-2.0 * log_sigmas.astype(np.float64))          # (O,)
    m2 = np.sum(mus.astype(np.float64) ** 2, axis=1)                # (O,)
    W = np.empty((K, O), np.float32)
    W[:D] = (2.0 * a[None, :] * mus.T.astype(np.float64)).astype(np.float32)
    W[D] = (-a).astype(np.float32)
    W[D + 1] = (-a * m2).astype(np.float32)

    x2 = np.sum(x * x, axis=-1)                                     # (B,S)
    in_maps = []
    for i in range(B):
        xa = np.empty((S, K), np.float32)
        xa[:, :D] = x[i]
        xa[:, D] = x2[i]
        xa[:, D + 1] = 1.0
        in_maps.append({"xaT": np.ascontiguousarray(xa.T), "w": W})

    nc = _build()
    global LAST_RESULT
    LAST_RESULT = res = bass_utils.run_bass_kernel_spmd(
        nc, in_maps, list(range(B)), **RUN_KWARGS
    )
    return np.stack([r["out"].astype(np.float32) for r in res.results], axis=0)


LAST_RESULT = None
RUN_KWARGS: dict = {}
